# revision 15
# baseline (speedup 1.0000x reference)
import numpy as np

# nn_Attention_77876347011151 — full-input Bass kernel, 8 NeuronCores,
# data-parallel over batch (1 batch per core).
#
# Shapes (hardcoded per spec): x [8,1025,768], alibi [1,12,1025,1025],
# coords [8,1024,2], mask [8,1025], gamma/beta [768], W_qkv [768,2304],
# W_out [768,768].
#
# Per-core math (batch b):
#   LN(x) -> xn (bf16), xnT via PE transpose
#   qkv = xn @ W_qkv  (W_q pre-scaled by 1/sqrt(dh) on host)
#   2D-RoPE on q,k rows 1.. (row 0 = CLS untouched), PE-transpose -> qT,kT
#   S^T[j,i] = kT.T @ qT (per head), += alibi^T (DVE add, alibi DMA'd to SBUF)
#   P^T = exp(S^T)  (no max-subtraction; values are small, validated)
#   mask folded into V-hat: V rows scaled by mask, extra column = mask
#   O[i,:] = P^T.T @ V-hat  -> O[:, :64]/O[:, 64]  (softmax normalization)
#   out = O_all @ W_out
B = 8
N = 1025
NP = 1152  # 9*128, n padded
IP = 1028  # i padded so S free-dim chunks are [512, 512, 4]
D = 768
H = 12
DH = 64
HALF = 32
C3 = 2304
ROPE_BASE = 8192.0
LN_EPS = 1e-5
NT = NP // 128  # 9 row tiles
JT_FULL = 8    # full 128-row j tiles; j=1024 handled as a 1-row special case

_CACHE = {}


def _build_nc():
    import concourse.bass as bass
    from concourse import bacc
    import concourse.mybir as mybir
    import concourse.tile as tile
    from concourse.masks import make_identity
    from contextlib import ExitStack

    f32 = mybir.dt.float32
    bf16 = mybir.dt.bfloat16
    AF = mybir.ActivationFunctionType
    OP = mybir.AluOpType

    nc = bacc.Bacc()
    x_d = nc.declare_dram_parameter("x", [NP, D], f32, isOutput=False)
    at_d = nc.declare_dram_parameter("alibiT", [H, N, IP], f32, isOutput=False)
    cos_d = nc.declare_dram_parameter("cosn", [128, NT, HALF], f32, isOutput=False)
    sin_d = nc.declare_dram_parameter("sinn", [128, NT, HALF], f32, isOutput=False)
    mm_d = nc.declare_dram_parameter("maskm", [128, NT], f32, isOutput=False)
    gam_d = nc.declare_dram_parameter("gamma", [128, 6], f32, isOutput=False)
    bet_d = nc.declare_dram_parameter("beta", [128, 6], f32, isOutput=False)
    wqkv_d = nc.declare_dram_parameter("wqkv", [128, 6 * C3], f32, isOutput=False)
    wout_d = nc.declare_dram_parameter("wout", [128, 6 * D], f32, isOutput=False)
    out_d = nc.declare_dram_parameter("out", [NP, D], f32, isOutput=True)

    with tile.TileContext(nc) as tc, ExitStack() as ctx:
        consts = ctx.enter_context(tc.tile_pool(name="consts", bufs=1))
        persist = ctx.enter_context(tc.tile_pool(name="persist", bufs=1))
        wsp = ctx.enter_context(tc.tile_pool(name="wsp", bufs=8))
        alrp = ctx.enter_context(tc.tile_pool(name="alrp", bufs=2))
        alp = ctx.enter_context(tc.tile_pool(name="alp", bufs=8))
        natp = ctx.enter_context(tc.tile_pool(name="natp", bufs=2))
        ppool = ctx.enter_context(tc.tile_pool(name="ppool", bufs=2))
        small = ctx.enter_context(tc.tile_pool(name="small", bufs=4))
        outp = ctx.enter_context(tc.tile_pool(name="outp", bufs=2))
        otb = ctx.enter_context(tc.tile_pool(name="otb", bufs=6))
        ps_big = ctx.enter_context(tc.tile_pool(name="ps_big", bufs=2, space="PSUM"))
        ps_s = ctx.enter_context(tc.tile_pool(name="ps_s", bufs=3, space="PSUM"))
        ps_o = ctx.enter_context(tc.tile_pool(name="ps_o", bufs=2, space="PSUM"))
        ps_t = ctx.enter_context(tc.tile_pool(name="ps_t", bufs=1, space="PSUM"))

        # ---- constants ----
        ident = consts.tile([128, 128], bf16)
        make_identity(nc, ident)
        eps_sb = consts.tile([128, 1], f32)
        nc.vector.memset(eps_sb, LN_EPS)
        cos_sb = consts.tile([128, NT, HALF], f32)
        nc.sync.dma_start(cos_sb, cos_d[:])
        sin_sb = consts.tile([128, NT, HALF], f32)
        nc.sync.dma_start(sin_sb, sin_d[:])
        mm_sb = consts.tile([128, NT], f32)
        nc.sync.dma_start(mm_sb, mm_d[:])
        gamT = consts.tile([128, 6], f32)
        nc.sync.dma_start(gamT, gam_d[:])
        betT = consts.tile([128, 6], f32)
        nc.sync.dma_start(betT, bet_d[:])
        # weights, cast to bf16 in flat chunks
        w_flat = persist.tile([128, 6 * C3], bf16)
        for cc in range(32):
            ws = wsp.tile([128, 432], f32, tag="ws", name="ws")
            nc.sync.dma_start(ws, wqkv_d[:, cc * 432 : (cc + 1) * 432])
            nc.vector.tensor_copy(out=w_flat[:, cc * 432 : (cc + 1) * 432], in_=ws)
        w_sb = w_flat.rearrange("p (o c) -> p o c", o=6)
        wo_flat = persist.tile([128, 6 * D], bf16)
        for cc in range(12):
            ws = wsp.tile([128, 432], f32, tag="ws", name="ws")[:, :384]
            nc.sync.dma_start(ws, wout_d[:, cc * 384 : (cc + 1) * 384])
            nc.vector.tensor_copy(out=wo_flat[:, cc * 384 : (cc + 1) * 384], in_=ws)
        wo_sb = wo_flat.rearrange("p (o c) -> p o c", o=6)

        # ---- persistent activations ----
        xnT = persist.tile([128, 6, NP], bf16)   # [d%128, d//128, n]
        qT = persist.tile([128, 6, NP], bf16)    # [64*(h%2)+c, h//2, n]
        kT = persist.tile([128, 6, NP], bf16)
        vhat = persist.tile([128, NT, H, DH + 1], bf16)  # [j%128, j//128, h, c]
        o_nat = persist.tile([128, NT, H, DH], bf16)     # [i%128, i//128, h, c]

        # ---- phase 1: LayerNorm + xnT ----
        for nt in range(NT):
            xta = alp.tile([128, 512], f32, tag="al", name="xta")
            nc.sync.dma_start(xta, x_d[nt * 128 : (nt + 1) * 128, 0:512])
            xtb = alp.tile([128, 512], f32, tag="al", name="xtb")[:, :256]
            nc.sync.dma_start(xtb, x_d[nt * 128 : (nt + 1) * 128, 512:768])
            st = small.tile([128, 3, 6], f32, tag="bn")
            nc.vector.bn_stats(out=st[:, 0, :], in_=xta[:, 0:256])
            nc.vector.bn_stats(out=st[:, 1, :], in_=xta[:, 256:512])
            nc.vector.bn_stats(out=st[:, 2, :], in_=xtb)
            mv = small.tile([128, 2], f32, tag="mv")
            nc.vector.bn_aggr(out=mv, in_=st)
            rs = small.tile([128, 1], f32, tag="rs")
            nc.scalar.activation(out=rs, in_=mv[:, 1:2], func=AF.Sqrt, bias=eps_sb, scale=1.0)
            rsr = small.tile([128, 1], f32, tag="rsr")
            nc.vector.reciprocal(out=rsr, in_=rs)
            xnb = natp.tile([128, D], bf16, tag="xnb")
            nc.vector.tensor_scalar(
                out=xnb[:, 0:512], in0=xta, scalar1=mv[:, 0:1], scalar2=rsr,
                op0=OP.subtract, op1=OP.mult,
            )
            nc.vector.tensor_scalar(
                out=xnb[:, 512:768], in0=xtb, scalar1=mv[:, 0:1], scalar2=rsr,
                op0=OP.subtract, op1=OP.mult,
            )
            for o in range(6):
                tp = ps_t.tile([128, 128], bf16, tag="tp")
                nc.tensor.transpose(tp, xnb[:, o * 128 : (o + 1) * 128], ident)
                # xnT = tp * gamma + beta   (per-partition scalars in this layout)
                nc.scalar.activation(
                    out=xnT[:, o, nt * 128 : (nt + 1) * 128], in_=tp,
                    func=AF.Identity, bias=betT[:, o : o + 1], scale=gamT[:, o : o + 1],
                )

        # ---- phase 2: QKV projection + RoPE + V-hat ----
        CHUNKS = [(0, 512), (512, 512), (1024, 512), (1536, 512), (2048, 256)]
        for nt in range(NT):
            qnat = natp.tile([128, D], bf16, tag="qnat")
            knat = natp.tile([128, D], bf16, tag="knat")
            for c0, cw in CHUNKS:
                ps = ps_big.tile([128, 512], f32, tag="mm", name="mmps")[:, :cw]
                for o in range(6):
                    nc.tensor.matmul(
                        ps, lhsT=xnT[:, o, nt * 128 : (nt + 1) * 128],
                        rhs=w_sb[:, o, c0 : c0 + cw],
                        start=(o == 0), stop=(o == 5),
                    )
                if c0 < 1536:
                    # rope ranges: (psum col offset, dest, dest col offset, nheads)
                    if c0 == 0:
                        rngs = [(0, qnat, 0, 8)]
                    elif c0 == 512:
                        rngs = [(0, qnat, 512, 4), (256, knat, 0, 4)]
                    else:
                        rngs = [(0, knat, 256, 8)]
                    for pc, dest, dc, nh in rngs:
                        pv = ps[:, pc : pc + nh * 64].rearrange("p (h c) -> p h c", c=64)
                        dv = dest[:, dc : dc + nh * 64].rearrange("p (h c) -> p h c", c=64)
                        t1, t2 = pv[:, :, 0:64:2], pv[:, :, 1:64:2]
                        cosb = cos_sb[:, nt, None, :].to_broadcast((128, nh, HALF))
                        sinb = sin_sb[:, nt, None, :].to_broadcast((128, nh, HALF))
                        ta = small.tile([128, 8, HALF], f32, tag="ta", name="ta")[:, :nh]
                        tb = small.tile([128, 8, HALF], f32, tag="tb", name="tb")[:, :nh]
                        nc.vector.tensor_tensor(ta, t1, cosb, OP.mult)
                        nc.vector.tensor_tensor(tb, t2, sinb, OP.mult)
                        nc.vector.tensor_tensor(dv[:, :, 0:HALF], ta, tb, OP.subtract)
                        nc.vector.tensor_tensor(ta, t1, sinb, OP.mult)
                        nc.vector.tensor_tensor(tb, t2, cosb, OP.mult)
                        nc.vector.tensor_tensor(dv[:, :, HALF:DH], ta, tb, OP.add)
                        if nt == 0:
                            # CLS row keeps original (un-rotated, un-permuted) values
                            nc.vector.tensor_copy(
                                out=dv[0:1].rearrange("p h c -> p (h c)"),
                                in_=ps[0:1, pc : pc + nh * 64],
                            )
                else:
                    h0, nh = (0, 8) if c0 == 1536 else (8, 4)
                    nc.vector.tensor_scalar_mul(
                        out=vhat[:, nt, h0 : h0 + nh, 0:DH],
                        in0=ps[:, :cw].rearrange("p (h c) -> p h c", c=64),
                        scalar1=mm_sb[:, nt : nt + 1],
                    )
            nc.vector.tensor_copy(
                out=vhat[:, nt, :, DH : DH + 1],
                in_=mm_sb[:, nt : nt + 1, None].to_broadcast((128, H, 1)),
            )
            for o in range(6):
                tp = ps_t.tile([128, 128], bf16, tag="tp")
                nc.tensor.transpose(tp, qnat[:, o * 128 : (o + 1) * 128], ident)
                nc.vector.tensor_copy(out=qT[:, o, nt * 128 : (nt + 1) * 128], in_=tp)
                tp2 = ps_t.tile([128, 128], bf16, tag="tp")
                nc.tensor.transpose(tp2, knat[:, o * 128 : (o + 1) * 128], ident)
                nc.vector.tensor_copy(out=kT[:, o, nt * 128 : (nt + 1) * 128], in_=tp2)

        # ---- phase 3: attention ----
        ICH = [(0, 512), (512, 512), (1024, 4)]
        for h in range(H):
            g, off = h // 2, (h % 2) * 64
            alr = alrp.tile([1, IP], f32, tag="alr")
            nc.sync.dma_start(alr, at_d[h, N - 1 : N, :])
            pt = ppool.tile([128, NT, NP], bf16, tag="pt")
            nc.vector.memset(pt[:, JT_FULL, :], 0.0)
            for jt in range(JT_FULL):
                for i0, iw in ICH:
                    ps = ps_s.tile([128, 512], f32, tag="s", name="sps")[:, :iw]
                    al = alp.tile([128, 512], f32, tag="al", name="al")[:, :iw]
                    nc.sync.dma_start(al, at_d[h, jt * 128 : (jt + 1) * 128, i0 : i0 + iw])
                    nc.tensor.matmul(
                        ps, lhsT=kT[off : off + 64, g, jt * 128 : (jt + 1) * 128],
                        rhs=qT[off : off + 64, g, i0 : i0 + iw],
                        start=True, stop=True,
                    )
                    nc.vector.tensor_tensor(ps, ps, al, OP.add)
                    nc.scalar.activation(
                        out=pt[:, jt, i0 : i0 + iw], in_=ps, func=AF.Exp
                    )
            # j = 1024 (single row)
            for i0, iw in ICH:
                ps = ps_s.tile([128, 512], f32, tag="s", name="sps1")[:1, :iw]
                nc.tensor.matmul(
                    ps, lhsT=kT[off : off + 64, g, 1024:1025],
                    rhs=qT[off : off + 64, g, i0 : i0 + iw],
                    start=True, stop=True,
                )
                nc.vector.tensor_tensor(ps, ps, alr[0:1, i0 : i0 + iw], OP.add)
                nc.scalar.activation(out=pt[0:1, JT_FULL, i0 : i0 + iw], in_=ps, func=AF.Exp)
            # P^T.T @ V-hat
            for it in range(NT):
                po = ps_o.tile([128, DH + 1], f32, tag="o")
                for jt in range(NT):
                    nc.tensor.matmul(
                        po, lhsT=pt[:, jt, it * 128 : (it + 1) * 128],
                        rhs=vhat[:, jt, h, :],
                        start=(jt == 0), stop=(jt == NT - 1),
                    )
                rl = small.tile([128, 1], f32, tag="rl")
                nc.vector.reciprocal(out=rl, in_=po[:, DH : DH + 1])
                nc.scalar.activation(
                    out=o_nat[:, it, h, :], in_=po[:, 0:DH], func=AF.Copy, scale=rl
                )

        # ---- phase 4: output projection ----
        for nt in range(NT):
            obl = []
            for o in range(6):
                tp = ps_t.tile([128, 128], bf16, tag="tp")
                nc.tensor.transpose(
                    tp,
                    o_nat[:, nt, 2 * o : 2 * o + 2, :].rearrange("p h c -> p (h c)"),
                    ident,
                )
                ob = otb.tile([128, 128], bf16, tag="ob")
                nc.vector.tensor_copy(out=ob, in_=tp)
                obl.append(ob)
            for e0, ew in [(0, 512), (512, 256)]:
                ps = ps_big.tile([128, 512], f32, tag="mm", name="mmps")[:, :ew]
                for o in range(6):
                    nc.tensor.matmul(
                        ps, lhsT=obl[o], rhs=wo_sb[:, o, e0 : e0 + ew],
                        start=(o == 0), stop=(o == 5),
                    )
                osb = outp.tile([128, 512], f32, tag="osb", name="osb")[:, :ew]
                nc.scalar.copy(out=osb, in_=ps)
                nc.sync.dma_start(out_d[nt * 128 : (nt + 1) * 128, e0 : e0 + ew], osb)
    nc.finalize()
    return nc


def _host_prep(x, alibi_bias, coords, mask, gamma, beta, W_qkv, W_out):
    inv = 1.0 / (ROPE_BASE ** (np.arange(HALF, dtype=np.float32) / HALF))
    wq = np.ascontiguousarray(W_qkv, dtype=np.float32).copy()
    wq[:, :D] *= DH ** -0.5
    aT = np.zeros((H, N, IP), np.float32)
    aT[:, :, :N] = alibi_bias[0].transpose(0, 2, 1)

    def stripe(v, inner):  # [(o p), ...] -> [p, o*inner...] flattened per partition
        o = v.shape[0] // 128
        return np.ascontiguousarray(
            v.reshape(o, 128, -1).transpose(1, 0, 2).reshape(128, -1)
        )

    shared = {
        "alibiT": aT,
        "gamma": stripe(np.asarray(gamma, np.float32), 1),
        "beta": stripe(np.asarray(beta, np.float32), 1),
        "wqkv": stripe(wq, C3),
        "wout": stripe(np.asarray(W_out, np.float32), D),
    }
    in_maps = []
    for b in range(B):
        fr = (coords[b, :, 0:1] + coords[b, :, 1:2]) * inv[None, :]
        cosP = np.zeros((NP, HALF), np.float32)
        sinP = np.zeros((NP, HALF), np.float32)
        cosP[1:N] = np.cos(fr)
        sinP[1:N] = np.sin(fr)
        cosP[0] = 1.0
        xP = np.zeros((NP, D), np.float32)
        xP[:N] = x[b]
        mmv = np.zeros((NP,), np.float32)
        mmv[:N] = mask[b].astype(np.float32)
        cosS = np.ascontiguousarray(cosP.reshape(NT, 128, HALF).transpose(1, 0, 2))
        sinS = np.ascontiguousarray(sinP.reshape(NT, 128, HALF).transpose(1, 0, 2))
        mmS = np.ascontiguousarray(mmv.reshape(NT, 128).T)
        in_maps.append({"x": xP, "cosn": cosS, "sinn": sinS, "maskm": mmS, **shared})
    return in_maps


def kernel(x, alibi_bias, coords, mask, gamma, beta, W_qkv, W_out, _want_trace=False):
    from concourse.bass_utils import run_bass_kernel_spmd

    x = np.asarray(x, dtype=np.float32)
    alibi_bias = np.asarray(alibi_bias, dtype=np.float32)
    coords = np.asarray(coords, dtype=np.float32)
    mask = np.asarray(mask)
    in_maps = _host_prep(x, alibi_bias, coords, mask, gamma, beta, W_qkv, W_out)
    if "nc" not in _CACHE:
        _CACHE["nc"] = _build_nc()
    nc = _CACHE["nc"]
    res = run_bass_kernel_spmd(nc, in_maps, core_ids=list(range(B)), trace=_want_trace)
    _CACHE["last"] = res
    out = np.empty((B, N, D), dtype=np.float32)
    for b in range(B):
        out[b] = res.results[b]["out"][:N]
    return out
